# revision 114
# baseline (speedup 1.0000x reference)
"""Trainium2 Bass kernel for BottleneckedEnsembleAttention.

Sharding: 8 cores, core c handles heads [2c, 2c+1] for both batches
(4 independent (b, head) attention problems per core).

Host precomputes: X pre-transposed to [128, 8, T] bf16 per (b,h) (no
on-chip transposes), cos/sin tables fp16 [128, T] (q rows 0-63, k rows
64-127, identical halves; attention scale folded into Wq), packed
bf16 weights, signed rotate-half permutation matrix P (rot_half via a
single 128-contraction matmul instead of a second projection pass),
causal upper-tri mask tile, active-mask bias rows.

Per (b, h) on-device pipeline:
  1. qk pass: pq = [Wq'|Wk]^T xT  -> psum [128, 512] per 512-col chunk
  2. rot = P @ pq_sb (one matmul); RoPE on DVE (add on GPSIMD):
     qkr = pq*cos + rot*sin; k rows DMA'd to partitions 0-63 (kq)
  3. v natural [t, 64] via bf16 matmuls (lhsT = xT tiles), plus a ones
     column for the softmax denominator
  4. attention per t-chunk (512 cols): scores^T = kq lhsT @ qkr rhs
     (f32r, causally column-narrowed), exp on ACT (bf16 out, narrowed)
     with per-partition active bias, causal tri multiply on GPSIMD
     (diagonal 128-blocks only), att^T accumulated over s-tiles in bf16
     (ones column gives the denominator in patt row 64; one open
     accumulation group per PSUM bank), o_proj bf16; per-t scale
     (active/denom) applied during PSUM eviction on DVE.
  5. output written bf16 in device tile order, host reshapes + converts.

The emission is software-pipelined: the next pair's loads + qk/rope,
this pair's deferred v tiles, and the previous chunk's o_proj finisher
are interleaved as filler steps inside the current chunk's attention
si-loop so the PE stays fed; ACT is kept almost exclusively for exp
since the per-si chain (scores -> exp -> mask -> att) paces attention.
"""

import math
from contextlib import ExitStack

import numpy as np
import ml_dtypes

import concourse.bass as bass
import concourse.mybir as mybir
import concourse.tile as tile
from concourse import bacc
from concourse.bass_utils import run_bass_kernel_spmd

# model constants (must match reference.py)
HIDDEN = 1024
HEADS = 16
HEAD_DIM = 64
THETA = 10000.0
TRAIN_LEN = 2048
SCALE = 4.0
ALPHA = 1.0
BETA = 32.0
B, T = 2, 2048

NCORES = 8
HPC = HEADS // NCORES  # heads per core = 2

F32 = mybir.dt.float32
F32R = mybir.dt.float32r
BF16 = mybir.dt.bfloat16
FP16 = mybir.dt.float16
BF = ml_dtypes.bfloat16

NEG_BIG = -1.0e30
DENOM_EPS = 1.0e-30

NT = T // 128   # 16 t-tiles of 128
NC4 = T // 512  # 4 chunks of 512
ND = HIDDEN // 128  # 8 d-chunks
NPAIR = B * HPC

EXPF = mybir.ActivationFunctionType.Exp


def _yarn_inv_freq_and_mscale():
    half = HEAD_DIM // 2
    pos_freqs = THETA ** (np.arange(half, dtype=np.float32) * 2.0 / HEAD_DIM)
    inv_freq_extra = (1.0 / pos_freqs).astype(np.float32)
    inv_freq_inter = (1.0 / (SCALE * pos_freqs)).astype(np.float32)

    def find_dim(num_rot):
        return (HEAD_DIM * math.log(TRAIN_LEN / (num_rot * 2.0 * math.pi))) / (
            2.0 * math.log(THETA)
        )

    low = max(math.floor(find_dim(BETA)), 0)
    high = min(math.ceil(find_dim(ALPHA)), half - 1)
    ramp = np.clip(
        (np.arange(half, dtype=np.float32) - low) / max(high - low, 1e-3), 0.0, 1.0
    ).astype(np.float32)
    extrap = (1.0 - ramp).astype(np.float32)
    inv_freq = inv_freq_inter * (1.0 - extrap) + inv_freq_extra * extrap
    mscale = 0.1 * math.log(SCALE) + 1.0 if SCALE > 1.0 else 1.0
    return inv_freq.astype(np.float32), np.float32(mscale)


def _host_prep(inputs):
    x = np.asarray(inputs["packed_embeddings"], dtype=np.float32)  # (B,16,T,H)
    pos = np.asarray(inputs["position_ids"])
    act = np.asarray(inputs["active_mask"])
    wq = np.asarray(inputs["q_proj"], dtype=np.float32)
    wk = np.asarray(inputs["k_proj"], dtype=np.float32)
    wv = np.asarray(inputs["v_proj"], dtype=np.float32)
    wo = np.asarray(inputs["o_proj"], dtype=np.float32)

    inv_freq, mscale = _yarn_inv_freq_and_mscale()
    scale = np.float32(mscale / math.sqrt(HEAD_DIM))

    ang = pos.astype(np.float32)[..., None] * inv_freq  # (B, 16, T, 32)
    cs = np.cos(ang).astype(np.float32)
    sn = np.sin(ang).astype(np.float32)
    cos64 = np.concatenate([cs, cs], axis=-1)  # (B, 16, T, 64)
    sin64 = np.concatenate([sn, sn], axis=-1)
    # transposed tables [B, 16, 128, T]: identical q/k halves (scale in Wq)
    cosT = np.ascontiguousarray(
        np.concatenate([cos64, cos64], axis=-1).transpose(0, 1, 3, 2)
    ).astype(np.float16)
    sinT = np.ascontiguousarray(
        np.concatenate([sin64, sin64], axis=-1).transpose(0, 1, 3, 2)
    ).astype(np.float16)

    # X^T folded: xt[b,l,p,c,t] = x[b,l,t,c*128+p]
    xt = x.transpose(0, 1, 3, 2).reshape(B, HEADS, ND, 128, T).transpose(0, 1, 3, 2, 4)
    xt = np.ascontiguousarray(xt).astype(BF)  # (B,16,128,8,T)

    wqk = np.concatenate([wq * scale, wk], axis=-1)  # (16,1024,128)
    wqk = np.ascontiguousarray(
        wqk.reshape(HEADS, ND, 128, 128).transpose(0, 2, 1, 3)
    ).astype(BF)  # (16,128,8,128)
    wvr = np.ascontiguousarray(
        wv.reshape(HEADS, ND, 128, HEAD_DIM).transpose(0, 2, 1, 3)
    ).astype(BF)  # (16,128,8,64)
    wob = np.ascontiguousarray(wo).astype(BF)  # (16,64,1024)

    actf = act.astype(np.float32)  # (B, 16, T)
    # bias rows for exp: 0 where active, -1e30 where inactive; [B,16,128,NT]
    actb = ((actf - 1.0) * (-NEG_BIG)).reshape(B, HEADS, NT, 128).transpose(0, 1, 3, 2)
    actb = np.ascontiguousarray(actb, dtype=np.float32)
    act01 = np.ascontiguousarray(
        actf.reshape(B, HEADS, NT, 128).transpose(0, 1, 3, 2), dtype=np.float32
    )

    # signed rotate-half permutation (lhsT): out[i] = sum_p P[p,i] in[p]
    P = np.zeros((128, 128), dtype=np.float32)
    for base in (0, 64):
        for u in range(32):
            P[base + u + 32, base + u] = -1.0  # out[u]    = -in[u+32]
            P[base + u, base + u + 32] = 1.0   # out[u+32] = +in[u]
    tri = np.triu(np.ones((128, 128), dtype=np.float32)).astype(BF)
    return dict(xt=xt, cosT=cosT, sinT=sinT, wqk=wqk, wv=wvr, wo=wob,
                actb=actb, act01=act01, P=P, tri=tri)


def _build_program():
    nc = bacc.Bacc("TRN2", target_bir_lowering=False, debug=False)

    x_d = nc.declare_dram_parameter("x", [B, HPC, 128, ND, T], BF16, isOutput=False)
    cos_d = nc.declare_dram_parameter("cos", [B, HPC, 128, T], FP16, isOutput=False)
    sin_d = nc.declare_dram_parameter("sin", [B, HPC, 128, T], FP16, isOutput=False)
    wqk_d = nc.declare_dram_parameter("wqk", [HPC, 128, ND, 128], BF16, isOutput=False)
    wv_d = nc.declare_dram_parameter("wv", [HPC, 128, ND, HEAD_DIM], BF16, isOutput=False)
    wo_d = nc.declare_dram_parameter("wo", [HPC, HEAD_DIM, HIDDEN], BF16, isOutput=False)
    actb_d = nc.declare_dram_parameter("actb", [128, NPAIR * NT], F32, isOutput=False)
    act01_d = nc.declare_dram_parameter("act01", [128, NPAIR * NT], F32, isOutput=False)
    p_d = nc.declare_dram_parameter("prot", [128, 128], F32R, isOutput=False)
    tri_d = nc.declare_dram_parameter("tri", [128, 128], BF16, isOutput=False)
    out_d = nc.declare_dram_parameter(
        "out", [B, HPC, NC4, 4, 128, HIDDEN], BF16, isOutput=True
    )

    with ExitStack() as ctx:
        tc = ctx.enter_context(tile.TileContext(nc))
        _emit(ctx, tc, nc, x_d, cos_d, sin_d, wqk_d, wv_d, wo_d,
              actb_d, act01_d, p_d, tri_d, out_d)
    nc.compile()
    return nc


def _emit(ctx, tc, nc, x_d, cos_d, sin_d, wqk_d, wv_d, wo_d,
          actb_d, act01_d, p_d, tri_d, out_d):
    # ---- pools ----
    consts = ctx.enter_context(tc.tile_pool(name="consts", bufs=1))
    wpool = ctx.enter_context(tc.tile_pool(name="wpool", bufs=1))
    xtp = ctx.enter_context(tc.tile_pool(name="xt", bufs=2))
    cssp = ctx.enter_context(tc.tile_pool(name="css", bufs=2))
    qkrp = ctx.enter_context(tc.tile_pool(name="qkr", bufs=2))
    kqp = ctx.enter_context(tc.tile_pool(name="kq", bufs=2))
    tmpp = ctx.enter_context(tc.tile_pool(name="tmps", bufs=2))
    vnp = ctx.enter_context(tc.tile_pool(name="vn", bufs=2))
    probp = ctx.enter_context(tc.tile_pool(name="prob", bufs=8))
    attp = ctx.enter_context(tc.tile_pool(name="att", bufs=2))
    rap = ctx.enter_context(tc.tile_pool(name="ra", bufs=2))
    outp = ctx.enter_context(tc.tile_pool(name="outsb", bufs=2))

    pjv = ctx.enter_context(tc.tile_pool(name="psum_pj", bufs=2, space="PSUM"))
    scp = ctx.enter_context(tc.tile_pool(name="psum_sc", bufs=3, space="PSUM"))
    atq = ctx.enter_context(tc.tile_pool(name="psum_att", bufs=1, space="PSUM"))
    opq = ctx.enter_context(tc.tile_pool(name="psum_o", bufs=2, space="PSUM"))

    # ---- constants / weights (once); DMAs issued in first-use order so
    # pair 0 starts as early as possible ----
    w = {h: {} for h in range(HPC)}

    def _w_tile(h, key, shape, dram):
        t = wpool.tile(shape, BF16, tag=f"w{key}{h}", name=f"w{key}{h}")
        nc.sync.dma_start(out=t, in_=dram[h])
        w[h][key] = t

    _w_tile(0, "qk", [128, ND, 128], wqk_d)

    pairs = [(b, h) for b in range(B) for h in range(HPC)]
    st = {}
    pending = []  # deferred chunk-finisher generators

    # ---------- phase emitters ----------
    def emit_loads(idx, xt_chunks=(0, 1, 2, 3)):
        b, h = pairs[idx]
        s = st.get(idx)
        if s is None:
            s = st[idx] = {}
            s["xt"] = xtp.tile([128, ND, T], BF16, tag="xt", name="xt")
            s["vn"] = vnp.tile([128, NT, 66], BF16, tag="vn", name="vn")
            nc.vector.memset(s["vn"][:, :, 64:65], 1.0)
            s["qkr"] = qkrp.tile([128, T], F32R, tag="qkr", name="qkr")
            s["kq"] = kqp.tile([64, T], F32R, tag="kq", name="kq")
        for g in xt_chunks:
            tsl = slice(g * 512, (g + 1) * 512)
            # split by d-halves: the first 4 accumulation matmuls of this
            # chunk can start after half the transfer
            nc.sync.dma_start(out=s["xt"][:, 0:4, tsl],
                              in_=x_d[b, h, :, 0:4, tsl])
            nc.sync.dma_start(out=s["xt"][:, 4:8, tsl],
                              in_=x_d[b, h, :, 4:8, tsl])
            if "cos" not in s:
                s["cos"] = cssp.tile([128, T], FP16, tag="cos", name="cos_sb")
                nc.sync.dma_start(out=s["cos"], in_=cos_d[b, h])
            elif "sin" not in s:
                s["sin"] = cssp.tile([128, T], FP16, tag="sin", name="sin_sb")
                nc.sync.dma_start(out=s["sin"], in_=sin_d[b, h])

    def emit_proj_steps(idx, bare=False):
        # qk projections + RoPE + v; one yield per small step. The `bare`
        # order hides rope-chain latency behind the v matmuls (used for
        # pair 0, which runs without attention filler around it).
        b, h = pairs[idx]
        s = st[idx]
        xt, qkr, kq = s["xt"], s["qkr"], s["kq"]

        def gen():
            for c in range(NC4):
                tsl = slice(c * 512, (c + 1) * 512)
                pq = pjv.tile([128, 512], F32, tag="pj", name="pq")
                for dc in range(ND):
                    nc.tensor.matmul(pq, lhsT=w[h]["qk"][:, dc, :],
                                     rhs=xt[:, dc, tsl],
                                     start=(dc == 0), stop=(dc == ND - 1))
                    if dc == 3:
                        yield
                psb = tmpp.tile([128, 512], F32R, tag="psb", name="psb")
                nc.vector.tensor_copy(psb, pq)
                if not bare:
                    yield
                qkc = tmpp.tile([128, 512], F32, tag="qkc", name="qkc")
                nc.vector.tensor_mul(qkc, pq, s["cos"][:, tsl])

                def emit_v():
                    # during pair-0's bare proj the score ring is idle;
                    # using it keeps the pj ring free for pq/rot rotation
                    vpool, vtag = (scp, "sc") if bare else (pjv, "pj")
                    pv = vpool.tile([128, 512], F32, tag=vtag, name="pv")
                    for t4 in range(4):
                        ti = 4 * c + t4
                        for dc in range(ND):
                            nc.tensor.matmul(
                                pv[:, 64 * t4:64 * t4 + 64],
                                lhsT=xt[:, dc, ti * 128:(ti + 1) * 128],
                                rhs=w[h]["v"][:, dc, :],
                                start=(dc == 0), stop=(dc == ND - 1),
                                skip_group_check=True)
                        if t4 % 2 == 1:
                            yield
                    pv4 = bass.AP(tensor=pv.tensor, offset=pv.offset,
                                  ap=[pv.ap[0], [64, 4], [1, 64]])
                    nc.scalar.copy(s["vn"][:, 4 * c:4 * c + 4, 0:64], pv4)

                if bare:
                    # hide the psb eviction behind the v matmuls
                    yield from emit_v()
                rot = pjv.tile([128, 512], F32, tag="pj", name="rot")
                nc.tensor.matmul(rot, lhsT=p_sb, rhs=psb, start=True, stop=True)
                qks = tmpp.tile([128, 512], F32, tag="qks", name="qks")
                nc.vector.tensor_mul(qks, rot, s["sin"][:, tsl])
                nc.gpsimd.tensor_add(qkr[:, tsl], qkc, qks)
                # k rows down to partitions 0-63 for the scores lhsT
                nc.sync.dma_start(out=kq[:, tsl], in_=qkr[64:128, tsl])
                yield
                if not bare and c == 0:
                    # v for chunk 0 is needed at the start of attention;
                    # chunks 1-3 are deferred into this pair's own
                    # attention si-loops (emit_v_tail)
                    yield from emit_v()
                    yield
        return gen()

    def emit_v_tail(idx):
        b, h = pairs[idx]
        s = st[idx]
        xt = s["xt"]

        def gen():
            for c in range(1, NC4):
                pv = pjv.tile([128, 512], F32, tag="pj", name="pv")
                for t4 in range(4):
                    ti = 4 * c + t4
                    for dc in range(ND):
                        nc.tensor.matmul(
                            pv[:, 64 * t4:64 * t4 + 64],
                            lhsT=xt[:, dc, ti * 128:(ti + 1) * 128],
                            rhs=w[h]["v"][:, dc, :],
                            start=(dc == 0), stop=(dc == ND - 1),
                            skip_group_check=True)
                    if t4 % 2 == 1:
                        yield
                pv4 = bass.AP(tensor=pv.tensor, offset=pv.offset,
                              ap=[pv.ap[0], [64, 4], [1, 64]])
                nc.scalar.copy(s["vn"][:, 4 * c:4 * c + 4, 0:64], pv4)
                yield
        return gen()

    def make_fin_steps(idx, tcx, att_sb):
        # deferred o_proj finisher for one chunk, as a filler generator
        b, h = pairs[idx]
        last = (idx == NPAIR - 1 and tcx == NC4 - 1)

        def gen():
            yield
            # denominator row (patt row 64, evac'd into att_sb) -> column
            pdn = scp.tile([128, 1024], BF16, tag="sc", name="pdn")
            for k in range(4):
                nc.tensor.transpose(
                    out=pdn[:, 2 * k:2 * k + 1],
                    in_=att_sb[HEAD_DIM:HEAD_DIM + 1, k * 128:(k + 1) * 128],
                    identity=ones_bf[HEAD_DIM:HEAD_DIM + 1, :],
                )
            pdn4 = bass.AP(tensor=pdn.tensor, offset=pdn.offset,
                           ap=[pdn.ap[0], [2, 4]])
            ra = rap.tile([128, 4], F32, tag="ra", name="ra")
            nc.vector.tensor_scalar_add(ra, pdn4, DENOM_EPS)
            nc.vector.reciprocal(ra, ra)
            nc.vector.tensor_mul(
                ra, ra, act01_sb[:, idx * NT + tcx * 4: idx * NT + tcx * 4 + 4])
            yield
            osb = outp.tile([128, 4, HIDDEN], BF16, tag="osb", name="osb")
            for k in range(4):
                for dh in range(2):
                    # the very last finisher borrows the (idle) proj psum
                    # ring to double its eviction pipeline depth
                    if last and (2 * k + dh) % 2 == 1:
                        po = pjv.tile([128, 512], F32, tag="pj", name="po")
                    else:
                        po = opq.tile([128, 512], F32, tag="o", name="po")
                    nc.tensor.matmul(
                        po,
                        lhsT=att_sb[0:HEAD_DIM, k * 128:(k + 1) * 128],
                        rhs=w[h]["o"][:, dh * 512:(dh + 1) * 512],
                        start=True, stop=True)
                    dst = osb[:, k, dh * 512:(dh + 1) * 512]
                    if last and (2 * k + dh) % 2 == 1:
                        nc.scalar.mul(dst, po, ra[:, k:k + 1])
                    else:
                        nc.vector.tensor_scalar_mul(dst, po, ra[:, k:k + 1])
                    yield
                if k == 1:
                    nc.sync.dma_start(
                        out=out_d[b, h, tcx, 0:2].rearrange("k p d -> p k d"),
                        in_=osb[:, 0:2, :])
            nc.sync.dma_start(
                out=out_d[b, h, tcx, 2:4].rearrange("k p d -> p k d"),
                in_=osb[:, 2:4, :])
        return gen()

    rr_state = [0]

    def step_fillers(fillers):
        n = len(fillers)
        if n == 0:
            return
        start = rr_state[0]
        for j in range(n):
            f = fillers[(start + j) % n]
            if next(f, None) is not None:
                rr_state[0] = (start + j + 1) % n
                return

    def emit_attn_chunk(idx, tcx, fillers):
        s = st[idx]
        qkr, kq, vn = s["qkr"], s["kq"], s["vn"]
        n_s = 4 * (tcx + 1)
        patt = atq.tile([128, 512], F32, tag="att", name="patt")
        pts = []

        def att_mm(sj):
            pt, c0 = pts[sj]
            nc.tensor.matmul(patt[0:HEAD_DIM + 1, c0:512],
                             lhsT=vn[:, sj, 0:HEAD_DIM + 1],
                             rhs=pt[:, c0:512],
                             start=(sj == 0), stop=(sj == n_s - 1),
                             skip_group_check=True)

        for si in range(n_s):
            kd = si - 4 * tcx
            c0 = 0 if kd < 0 else 128 * kd
            c0s = 0 if kd < 0 else min(128 * kd, 256)
            psc = scp.tile([128, 512], F32, tag="sc", name="psc")
            nc.tensor.matmul(
                psc[:, c0s:512],
                lhsT=kq[:, si * 128:(si + 1) * 128],
                rhs=qkr[0:64, tcx * 512 + c0s:(tcx + 1) * 512],
                start=True, stop=True)
            pt = probp.tile([128, 512], BF16, tag="prob", name="pt")
            nc.scalar.activation(pt[:, c0:512], psc[:, c0:512], EXPF,
                                 bias=actb_sb[:, idx * NT + si:idx * NT + si + 1])
            if kd >= 0:
                nc.gpsimd.tensor_mul(pt[:, c0:c0 + 128], pt[:, c0:c0 + 128],
                                     tri_sb[:, 0:128])
            pts.append((pt, c0))
            # PE runs ahead early in a chunk and stalls late (exp latency
            # accumulates): consume filler mostly in the back half
            if si >= n_s // 2:
                step_fillers(fillers)
                if si >= n_s - 3:
                    step_fillers(fillers)
            if si >= 4:
                att_mm(si - 4)
        for sj in range(max(n_s - 4, 0), n_s):
            att_mm(sj)
        att_sb = attp.tile([128, 512], BF16, tag="attsb", name="att_sb")
        nc.scalar.copy(att_sb[0:HEAD_DIM + 1, :], patt[0:HEAD_DIM + 1, :])
        pending.append(make_fin_steps(idx, tcx, att_sb))

    # ---------- startup: remaining consts in first-use order ----------
    emit_loads(0, xt_chunks=(0,))
    p_sb = consts.tile([128, 128], F32R)
    nc.sync.dma_start(out=p_sb, in_=p_d[:, :])
    actb_sb = consts.tile([128, NPAIR * NT], F32)
    nc.sync.dma_start(out=actb_sb, in_=actb_d[:, :])
    tri_sb = consts.tile([128, 128], BF16)
    nc.sync.dma_start(out=tri_sb, in_=tri_d[:, :])
    ones_bf = consts.tile([128, 1], BF16)
    nc.vector.memset(ones_bf, 1.0)
    _w_tile(0, "v", [128, ND, HEAD_DIM], wv_d)
    emit_loads(0, xt_chunks=(1,))
    act01_sb = consts.tile([128, NPAIR * NT], F32)
    nc.sync.dma_start(out=act01_sb, in_=act01_d[:, :])
    _w_tile(0, "o", [HEAD_DIM, HIDDEN], wo_d)
    emit_loads(0, xt_chunks=(2, 3))
    _w_tile(1, "qk", [128, ND, 128], wqk_d)
    _w_tile(1, "v", [128, ND, HEAD_DIM], wv_d)
    _w_tile(1, "o", [HEAD_DIM, HIDDEN], wo_d)

    # ---------- interleaved pipeline across pairs ----------
    for _ in emit_proj_steps(0, bare=True):
        pass
    vtail = None  # pair 0's bare proj already emitted all its v chunks
    for idx in range(NPAIR):
        proj_fill = None
        for tcx in range(NC4):
            if tcx == 0 and idx + 1 < NPAIR:
                emit_loads(idx + 1)
                proj_fill = emit_proj_steps(idx + 1)
            fillers = list(pending)
            pending.clear()
            if proj_fill is not None and tcx >= 1:
                fillers.insert(0, proj_fill)
            if vtail is not None and tcx < NC4 - 1:
                fillers.append(vtail)
            emit_attn_chunk(idx, tcx, fillers)
            if vtail is not None and tcx < NC4 - 1:
                # this pair's v tiles for chunk tcx+1 must be complete
                # before that chunk's att matmuls
                for _ in range(3):
                    next(vtail, None)
            for f in fillers:
                if f is not proj_fill and f is not vtail:
                    for _ in f:
                        pass
        if vtail is not None:
            for _ in vtail:
                pass
        vtail = emit_v_tail(idx + 1) if idx + 1 < NPAIR else None
        if proj_fill is not None:
            for _ in proj_fill:
                pass
        if idx > 0:
            del st[idx - 1]
    while pending:
        for _ in pending.pop(0):
            pass


_PROGRAM = None


def _make_in_maps(prep):
    in_maps = []
    for c in range(NCORES):
        hs = slice(c * HPC, (c + 1) * HPC)
        actb = np.ascontiguousarray(
            prep["actb"][:, hs].transpose(2, 0, 1, 3).reshape(128, NPAIR * NT))
        act01 = np.ascontiguousarray(
            prep["act01"][:, hs].transpose(2, 0, 1, 3).reshape(128, NPAIR * NT))
        in_maps.append({
            "x": np.ascontiguousarray(prep["xt"][:, hs]),
            "cos": np.ascontiguousarray(prep["cosT"][:, hs]),
            "sin": np.ascontiguousarray(prep["sinT"][:, hs]),
            "wqk": np.ascontiguousarray(prep["wqk"][hs]),
            "wv": np.ascontiguousarray(prep["wv"][hs]),
            "wo": np.ascontiguousarray(prep["wo"][hs]),
            "actb": actb,
            "act01": act01,
            "prot": prep["P"],
            "tri": prep["tri"],
        })
    return in_maps


def kernel(**inputs) -> np.ndarray:
    global _PROGRAM
    prep = _host_prep(inputs)
    if _PROGRAM is None:
        _PROGRAM = _build_program()
    nc = _PROGRAM

    in_maps = _make_in_maps(prep)
    res = run_bass_kernel_spmd(nc, in_maps, list(range(NCORES)))
    outs = [
        np.asarray(res.results[c]["out"]).reshape(B, HPC, T, HIDDEN)
        for c in range(NCORES)
    ]
    return np.concatenate(outs, axis=1).astype(np.float32)
